# revision 14
# baseline (speedup 1.0000x reference)
"""Trainium2 Bass kernel for AMM (landmark/Nystrom-style) attention.

Per batch element (8 total, one NeuronCore each):
    qkv  = x @ W_qkv; q,k,v = split(qkv); q /= sqrt(512)
    keys_lm = segment_mean(k, 16); vals_lm = segment_mean(v, 16)
    out  = softmax(q @ keys_lm^T) @ vals_lm @ W_proj + b_proj
    return v + out

Algebraic restructuring (exact in real arithmetic):
  - segment_mean commutes with the projections: keys_lm = pool(x) @ W_k,
    vals_lm = pool(x) @ W_v  -> the full k matmul is never computed.
  - attn @ vals_lm @ W_proj -> attn @ (vals_lm @ W_proj): 256 rows through
    W_proj instead of 4096.
  - b_proj folded into VW by a rank-1 matmul (softmax rows sum to 1).
  - softmax normalization applied after the value matmul:
        out = (E @ VWb) / (E @ 1),  E = exp(logits).

Performance shape (HW-measured: the kernel is DMA- and PE-bound):
  - DMA is ~330 GB/s per core and near-serial across queues, with ~0.4us
    per dma_start overhead -> few, large transfers; inputs on the sync ring,
    outputs batched per 512-row chunk on the gpsimd ring.
  - x is shipped once, in bf16 (4MB). No fp8 inputs: extra HBM traffic
    costs more than fp8 matmuls save.
  - q/v/landmark matmuls run in bf16 (1 cycle/row).  The attention-phase
    matmuls (scores, out2, denominator) run fp8e4m3 + DoubleRow (2.2x
    faster, HW-measured) using operands produced ON DEVICE by the psum->
    sbuf copies that are needed anyway (qt, keysT, E=exp, VW).
  - v (the residual, dominates output accuracy) stays bf16: rel err ~3e-3
    against the fp32 reference (gate 2e-2).
  - landmark pooling: pairwise-add trees on the idle GpSimd engine for half
    the channel tiles, DVE segment-reduces for the other half.
"""

import sys
from contextlib import ExitStack

import numpy as np

sys.path.insert(0, "/opt/trn_rl_repo")

import concourse.bass as bass  # noqa: E402
import concourse.tile as tile  # noqa: E402
from concourse import bacc, mybir  # noqa: E402
from concourse.bass_utils import run_bass_kernel_spmd  # noqa: E402

import ml_dtypes  # noqa: E402

BF16 = mybir.dt.bfloat16
F8 = mybir.dt.float8e4
F32 = mybir.dt.float32
AF = mybir.ActivationFunctionType
ALU = mybir.AluOpType
DR = mybir.MatmulPerfMode.DoubleRow

B, N, DIM = 8, 4096, 512
L, SEG = 256, 16
CT = DIM // 128  # 4 channel partition tiles
MT = N // 512  # 8 row chunks
QT_S = 4.0  # qt = 4*q            (fp8 sweet spot)
KT_S = 1.0  # keysT = 16*keys_lm  (pool sums are 16*mean already)
VL_S = 1.0 / 16.0  # valsT = vals_lm
VW_S = 64.0  # vw = 64*(VW + 1b)
DEN_S = 64.0  # ones value; den psum = 64*sum(E) cancels VW_S via reciprocal
ESCALE = float(1.0 / np.sqrt(512.0) / 64.0)  # logits psum = 64*raw


def build_kernel(ctx: ExitStack, tc: "tile.TileContext", out_d, xt_d, wqkv_d, wproj_d, bproj_d):
    nc = tc.nc

    consts = ctx.enter_context(tc.tile_pool(name="consts", bufs=1))
    work = ctx.enter_context(tc.tile_pool(name="work", bufs=3))
    finpool = ctx.enter_context(tc.tile_pool(name="finpool", bufs=3))
    gwork = ctx.enter_context(tc.tile_pool(name="gwork", bufs=3))
    psum = ctx.enter_context(tc.tile_pool(name="psum", bufs=4, space="PSUM"))
    psumv = ctx.enter_context(tc.tile_pool(name="psumv", bufs=2, space="PSUM"))
    psden = ctx.enter_context(tc.tile_pool(name="psden", bufs=2, space="PSUM"))

    # ---- weights (single batched DMA each) ----------------------------------
    wqkv = consts.tile([128, CT, 3 * DIM], BF16)
    nc.sync.dma_start(out=wqkv[:, :, :], in_=wqkv_d.rearrange("j p w -> p j w"))
    wproj = consts.tile([128, CT, DIM], BF16)
    nc.sync.dma_start(out=wproj[:, :, :], in_=wproj_d.rearrange("j p w -> p j w"))
    bproj = consts.tile([1, DIM], BF16)
    nc.sync.dma_start(out=bproj[:, :], in_=bproj_d[:, :])

    ones_col = consts.tile([128, 2, 1], F8)
    nc.vector.memset(ones_col[:, :, :], DEN_S)
    ones_row = consts.tile([1, 128], BF16)
    nc.vector.memset(ones_row[:, :], 1.0)

    # ---- x^T in, 2048-wide; pooling split GpSimd trees / DVE reduces -------
    xt = consts.tile([128, CT, N], BF16)  # x^T
    xpool = consts.tile([128, CT, L], BF16)  # 16 * segment_mean(x)^T
    LC2 = 2 * L // MT  # 64 landmarks per 2048-chunk
    for hi in range(2):
        c0, c1 = hi * 2048, (hi + 1) * 2048
        for j in range(CT):
            nc.sync.dma_start(out=xt[:, j, c0:c1], in_=xt_d[j, :, c0:c1])
        for j in (0, 1):  # GpSimd pairwise-add tree: 2048 -> 128 segment sums
            cur = xt[:, j, c0:c1]
            for sz in (1024, 512, 256):
                dst = gwork.tile([128, sz], BF16, tag=f"tree{sz}")
                pair = cur.rearrange("p (a two) -> p a two", two=2)
                nc.gpsimd.tensor_add(dst[:, :], pair[:, :, 0], pair[:, :, 1])
                cur = dst
            pair = cur.rearrange("p (a two) -> p a two", two=2)
            nc.gpsimd.tensor_add(
                xpool[:, j, hi * LC2 * 2 : (hi + 1) * LC2 * 2],
                pair[:, :, 0],
                pair[:, :, 1],
            )
        for j in (2, 3):  # DVE segment reduce -> 16*mean
            pf = work.tile([128, 2 * LC2], F32, tag="poolf")
            nc.vector.reduce_sum(
                pf[:, :],
                xt[:, j, c0:c1].rearrange("p (l s) -> p l s", s=SEG),
                axis=mybir.AxisListType.X,
            )
            nc.vector.tensor_scalar_mul(
                xpool[:, j, hi * LC2 * 2 : (hi + 1) * LC2 * 2], pf[:, :], 1.0
            )

    # ---- q^T (bf16 -> fp8 via ACT copy) + v (bf16), interleaved ------------
    qtf8 = consts.tile([128, CT, N], F8)  # 4*q^T
    vstore = consts.tile([128, 32, 512], BF16)  # v, natural layout
    for mi in range(MT):
        for dj in range(CT):
            pt = psum.tile([128, 512], F32, tag="mm")
            for cj in range(CT):
                nc.tensor.matmul(
                    pt[:, :],
                    wqkv[:, cj, dj * 128 : (dj + 1) * 128],
                    xt[:, cj, mi * 512 : (mi + 1) * 512],
                    start=(cj == 0),
                    stop=(cj == CT - 1),
                )
            nc.scalar.mul(qtf8[:, dj, mi * 512 : (mi + 1) * 512], pt[:, :], QT_S)
        for t in range(4):
            r0 = mi * 512 + t * 128
            vp = psumv.tile([128, 512], F32, tag="mmv")
            for cj in range(CT):
                nc.tensor.matmul(
                    vp[:, :],
                    xt[:, cj, r0 : r0 + 128],
                    wqkv[:, cj, 2 * DIM : 3 * DIM],
                    start=(cj == 0),
                    stop=(cj == CT - 1),
                )
            if t % 2 == 0:  # balance psum->sbuf copies across DVE and ACT
                nc.vector.tensor_copy(vstore[:, mi * 4 + t, :], vp[:, :])
            else:
                nc.scalar.copy(vstore[:, mi * 4 + t, :], vp[:, :])

    # ---- landmark projections (bf16 matmuls; fp8/bf16 copies) ---------------
    keysT = consts.tile([128, CT, L], F8)  # 16*keys_lm^T
    valsT = consts.tile([128, CT, L], BF16)  # vals_lm^T
    for dst, col0, s in ((keysT, DIM, KT_S), (valsT, 2 * DIM, VL_S)):
        for dj in range(CT):
            pt = psum.tile([128, L], F32, tag="mm")
            for cj in range(CT):
                nc.tensor.matmul(
                    pt[:, :],
                    wqkv[:, cj, col0 + dj * 128 : col0 + (dj + 1) * 128],
                    xpool[:, cj, :],
                    start=(cj == 0),
                    stop=(cj == CT - 1),
                )
            nc.scalar.mul(dst[:, dj, :], pt[:, :], s)

    # ---- VWb = 64*(vals_lm @ W_proj + 1b)  [l_lo, li, d] fp8 ----------------
    vw = consts.tile([128, 2, DIM], F8)
    for li in range(2):
        pt = psum.tile([128, DIM], F32, tag="mm")
        for dj in range(CT):
            nc.tensor.matmul(
                pt[:, :],
                valsT[:, dj, li * 128 : (li + 1) * 128],
                wproj[:, dj, :],
                start=(dj == 0),
                stop=False,
            )
        nc.tensor.matmul(pt[:, :], ones_row[:, :], bproj[:, :], start=False, stop=True)
        nc.scalar.mul(vw[:, li, :], pt[:, :], VW_S)

    # ---- attention, all matmuls fp8 DoubleRow; batched output DMA ----------
    for mi in range(MT):
        et = work.tile([128, 2, 512], F8, tag="et")  # E = exp(logits/sqrt(512))
        for li in range(2):
            pt = psum.tile([128, 512], F32, tag="mm")
            for dr in range(2):
                nc.tensor.matmul(
                    pt[:, :],
                    keysT[:, 2 * dr : 2 * dr + 2, li * 128 : (li + 1) * 128],
                    qtf8[:, 2 * dr : 2 * dr + 2, mi * 512 : (mi + 1) * 512],
                    start=(dr == 0),
                    stop=(dr == 1),
                    perf_mode=DR,
                )
            nc.scalar.activation(et[:, li, :], pt[:, :], AF.Exp, scale=ESCALE)

        fin = finpool.tile([128, 4, 512], F32, tag="fin")
        for t in range(4):
            sl = slice(t * 128, (t + 1) * 128)
            dp = psden.tile([128, 1], F32, tag="den")
            nc.tensor.matmul(
                dp[:, :], et[:, :, sl], ones_col[:, :, :],
                start=True, stop=True, perf_mode=DR,
            )
            rr = work.tile([128, 1], F32, tag="rr")
            nc.vector.reciprocal(rr[:, :], dp[:, :])
            op = psum.tile([128, 512], F32, tag="mm")
            nc.tensor.matmul(
                op[:, :], et[:, :, sl], vw[:, :, :],
                start=True, stop=True, perf_mode=DR,
            )
            nc.vector.scalar_tensor_tensor(
                fin[:, t, :], op[:, :], rr[:, :], vstore[:, mi * 4 + t, :],
                op0=ALU.mult, op1=ALU.add,
            )
        nc.sync.dma_start(
            out=out_d[mi * 512 : (mi + 1) * 512, :].rearrange("(t p) d -> p t d", p=128),
            in_=fin[:, :, :],
        )


def build_nc(repeat: int = 1):
    nc = bacc.Bacc("TRN2", target_bir_lowering=False, debug=False, num_devices=8)
    xt_d = nc.declare_dram_parameter("xt", [CT, 128, N], BF16, isOutput=False)
    wqkv_d = nc.declare_dram_parameter("wqkv", [CT, 128, 3 * DIM], BF16, isOutput=False)
    wproj_d = nc.declare_dram_parameter("wproj", [CT, 128, DIM], BF16, isOutput=False)
    bproj_d = nc.declare_dram_parameter("bproj", [1, DIM], BF16, isOutput=False)
    out_d = nc.declare_dram_parameter("out", [N, DIM], F32, isOutput=True)
    aps = (out_d.ap(), xt_d.ap(), wqkv_d.ap(), wproj_d.ap(), bproj_d.ap())
    with tile.TileContext(nc) as tc, ExitStack() as ctx:
        if repeat == 1:
            build_kernel(ctx, tc, *aps)
        else:
            with tc.For_i(0, repeat, 1):
                build_kernel(ctx, tc, *aps)
    nc.compile()
    return nc


def prep_in_maps(x, W_qkv, W_proj, b_proj):
    bf = ml_dtypes.bfloat16
    wq = np.ascontiguousarray(np.asarray(W_qkv, np.float32).astype(bf).reshape(CT, 128, 3 * DIM))
    wp = np.ascontiguousarray(np.asarray(W_proj, np.float32).astype(bf).reshape(CT, 128, DIM))
    bp = np.asarray(b_proj, np.float32).astype(bf).reshape(1, DIM)
    in_maps = []
    for i in range(B):
        xti = np.ascontiguousarray(np.asarray(x[i], np.float32).T.astype(bf)).reshape(CT, 128, N)
        in_maps.append({"xt": xti, "wqkv": wq, "wproj": wp, "bproj": bp})
    return in_maps


_NC_CACHE = None


def kernel(x, W_qkv, W_proj, b_proj):
    global _NC_CACHE
    if _NC_CACHE is None:
        _NC_CACHE = build_nc()
    nc = _NC_CACHE
    in_maps = prep_in_maps(x, W_qkv, W_proj, b_proj)
    res = run_bass_kernel_spmd(nc, in_maps, core_ids=list(range(B)))
    out = np.stack([res.results[i]["out"] for i in range(B)], axis=0)
    return out.astype(np.float32)


# revision 18
# speedup vs baseline: 1.4252x; 1.4252x over previous
"""Trainium2 Bass kernel for AMM (landmark/Nystrom-style) attention.

Per batch element (8 total, one NeuronCore each):
    qkv  = x @ W_qkv; q,k,v = split(qkv); q /= sqrt(512)
    keys_lm = segment_mean(k, 16); vals_lm = segment_mean(v, 16)
    out  = softmax(q @ keys_lm^T) @ vals_lm @ W_proj + b_proj
    return v + out

Algebraic restructuring (exact in real arithmetic):
  - segment_mean commutes with the projections: keys_lm = pool(x) @ W_k,
    vals_lm = pool(x) @ W_v  -> the full k matmul is never computed.
  - attn @ vals_lm @ W_proj -> attn @ (vals_lm @ W_proj): 256 rows through
    W_proj instead of 4096.
  - b_proj folded into VW by a rank-1 matmul (softmax rows sum to 1).
  - softmax normalization applied after the value matmul:
        out = (E @ VWb) / (E @ 1),  E = exp(logits).

Performance shape (HW-measured: the kernel is DMA- and PE-bound):
  - DMA is ~330 GB/s per core and near-serial across queues, with ~0.4us
    per dma_start overhead -> few, large transfers; inputs on the sync ring,
    outputs batched per 512-row chunk on the gpsimd ring.
  - x is shipped once, in bf16 (4MB). No fp8 inputs: extra HBM traffic
    costs more than fp8 matmuls save.
  - q/v/landmark matmuls run in bf16 (1 cycle/row).  The attention-phase
    matmuls (scores, out2, denominator) run fp8e4m3 + DoubleRow (2.2x
    faster, HW-measured) using operands produced ON DEVICE by the psum->
    sbuf copies that are needed anyway (qt, keysT, E=exp, VW).
  - v (the residual, dominates output accuracy) stays bf16: rel err ~3e-3
    against the fp32 reference (gate 2e-2).
  - landmark pooling: pairwise-add trees on the idle GpSimd engine for half
    the channel tiles, DVE segment-reduces for the other half.
"""

import sys
from contextlib import ExitStack

import numpy as np

sys.path.insert(0, "/opt/trn_rl_repo")

import concourse.bass as bass  # noqa: E402
import concourse.tile as tile  # noqa: E402
from concourse import bacc, mybir  # noqa: E402
from concourse.bass_utils import run_bass_kernel_spmd  # noqa: E402

import ml_dtypes  # noqa: E402

BF16 = mybir.dt.bfloat16
F8 = mybir.dt.float8e4
F32 = mybir.dt.float32
AF = mybir.ActivationFunctionType
ALU = mybir.AluOpType
DR = mybir.MatmulPerfMode.DoubleRow

B, N, DIM = 8, 4096, 512
L, SEG = 256, 16
CT = DIM // 128  # 4 channel partition tiles
MT = N // 512  # 8 row chunks
QT_S = 4.0  # qt = 4*q            (fp8 sweet spot)
KT_S = 1.0  # keysT = 16*keys_lm  (pool sums are 16*mean already)
VL_S = 1.0 / 16.0  # valsT = vals_lm
VW_S = 64.0  # vw = 64*(VW + 1b)
DEN_S = 64.0  # ones value; den psum = 64*sum(E) cancels VW_S via reciprocal
ESCALE = float(1.0 / np.sqrt(512.0) / 64.0)  # logits psum = 64*raw


def build_kernel(ctx: ExitStack, tc: "tile.TileContext", out_d, xt_d, wqkv_d, wproj_d, bproj_d):
    nc = tc.nc

    consts = ctx.enter_context(tc.tile_pool(name="consts", bufs=1))
    work = ctx.enter_context(tc.tile_pool(name="work", bufs=3))
    finpool = ctx.enter_context(tc.tile_pool(name="finpool", bufs=3))
    gwork = ctx.enter_context(tc.tile_pool(name="gwork", bufs=3))
    psum = ctx.enter_context(tc.tile_pool(name="psum", bufs=4, space="PSUM"))
    psumv = ctx.enter_context(tc.tile_pool(name="psumv", bufs=2, space="PSUM"))
    psden = ctx.enter_context(tc.tile_pool(name="psden", bufs=2, space="PSUM"))

    # ---- weights ------------------------------------------------------------
    wqkv = consts.tile([128, CT, 3 * DIM], BF16)
    for j in range(CT):
        nc.sync.dma_start(out=wqkv[:, j, :], in_=wqkv_d[j, :, :])
    wproj = consts.tile([128, CT, DIM], BF16)
    for j in range(CT):
        nc.sync.dma_start(out=wproj[:, j, :], in_=wproj_d[j, :, :])
    bproj = consts.tile([1, DIM], BF16)
    nc.sync.dma_start(out=bproj[:, :], in_=bproj_d[:, :])

    ones_col = consts.tile([128, 2, 1], F8)
    nc.vector.memset(ones_col[:, :, :], DEN_S)
    ones_row = consts.tile([1, 128], BF16)
    nc.vector.memset(ones_row[:, :], 1.0)

    # ---- x^T in, 2048-wide chunks; DVE segment reduces for pooling ---------
    xt = consts.tile([128, CT, N], BF16)  # x^T
    xpool = consts.tile([128, CT, L], BF16)  # 16 * segment_mean(x)^T
    LH = L // 2  # 128 landmarks per 2048-chunk
    for hi in range(2):
        c0, c1 = hi * 2048, (hi + 1) * 2048
        for j in range(CT):
            nc.sync.dma_start(out=xt[:, j, c0:c1], in_=xt_d[j, :, c0:c1])
        for j in range(CT):  # DVE segment reduce -> 16*mean (segment sums)
            pf = work.tile([128, LH], F32, tag="poolf")
            nc.vector.reduce_sum(
                pf[:, :],
                xt[:, j, c0:c1].rearrange("p (l s) -> p l s", s=SEG),
                axis=mybir.AxisListType.X,
            )
            nc.vector.tensor_scalar_mul(
                xpool[:, j, hi * LH : (hi + 1) * LH], pf[:, :], 1.0
            )

    # ---- q^T (bf16 -> fp8 via ACT copy) + v (bf16), interleaved ------------
    qtf8 = consts.tile([128, CT, N], F8)  # 4*q^T
    vstore = consts.tile([128, 32, 512], BF16)  # v, natural layout
    for mi in range(MT):
        for dj in range(CT):
            pt = psum.tile([128, 512], F32, tag="mm")
            for cj in range(CT):
                nc.tensor.matmul(
                    pt[:, :],
                    wqkv[:, cj, dj * 128 : (dj + 1) * 128],
                    xt[:, cj, mi * 512 : (mi + 1) * 512],
                    start=(cj == 0),
                    stop=(cj == CT - 1),
                )
            nc.scalar.mul(qtf8[:, dj, mi * 512 : (mi + 1) * 512], pt[:, :], QT_S)
        for t in range(4):
            r0 = mi * 512 + t * 128
            vp = psumv.tile([128, 512], F32, tag="mmv")
            for cj in range(CT):
                nc.tensor.matmul(
                    vp[:, :],
                    xt[:, cj, r0 : r0 + 128],
                    wqkv[:, cj, 2 * DIM : 3 * DIM],
                    start=(cj == 0),
                    stop=(cj == CT - 1),
                )
            if t == 0:  # balance psum->sbuf copies: 1/4 on DVE, 3/4 on ACT
                nc.vector.tensor_copy(vstore[:, mi * 4 + t, :], vp[:, :])
            else:
                nc.scalar.copy(vstore[:, mi * 4 + t, :], vp[:, :])

    # ---- landmark projections (bf16 matmuls; fp8/bf16 copies) ---------------
    keysT = consts.tile([128, CT, L], F8)  # 16*keys_lm^T
    valsT = consts.tile([128, CT, L], BF16)  # vals_lm^T
    for dst, col0, s in ((keysT, DIM, KT_S), (valsT, 2 * DIM, VL_S)):
        for dj in range(CT):
            pt = psum.tile([128, L], F32, tag="mm")
            for cj in range(CT):
                nc.tensor.matmul(
                    pt[:, :],
                    wqkv[:, cj, col0 + dj * 128 : col0 + (dj + 1) * 128],
                    xpool[:, cj, :],
                    start=(cj == 0),
                    stop=(cj == CT - 1),
                )
            nc.scalar.mul(dst[:, dj, :], pt[:, :], s)

    # ---- VWb = 64*(vals_lm @ W_proj + 1b)  [l_lo, li, d] fp8 ----------------
    vw = consts.tile([128, 2, DIM], F8)
    for li in range(2):
        pt = psum.tile([128, DIM], F32, tag="mm")
        for dj in range(CT):
            nc.tensor.matmul(
                pt[:, :],
                valsT[:, dj, li * 128 : (li + 1) * 128],
                wproj[:, dj, :],
                start=(dj == 0),
                stop=False,
            )
        nc.tensor.matmul(pt[:, :], ones_row[:, :], bproj[:, :], start=False, stop=True)
        nc.scalar.mul(vw[:, li, :], pt[:, :], VW_S)

    # ---- attention, all matmuls fp8 DoubleRow; batched output DMA ----------
    for mi in range(MT):
        et = work.tile([128, 2, 512], F8, tag="et")  # E = exp(logits/sqrt(512))
        for li in range(2):
            pt = psum.tile([128, 512], F32, tag="mm")
            for dr in range(2):
                nc.tensor.matmul(
                    pt[:, :],
                    keysT[:, 2 * dr : 2 * dr + 2, li * 128 : (li + 1) * 128],
                    qtf8[:, 2 * dr : 2 * dr + 2, mi * 512 : (mi + 1) * 512],
                    start=(dr == 0),
                    stop=(dr == 1),
                    perf_mode=DR,
                )
            nc.scalar.activation(et[:, li, :], pt[:, :], AF.Exp, scale=ESCALE)

        for t in range(4):
            r0 = mi * 512 + t * 128
            sl = slice(t * 128, (t + 1) * 128)
            dp = psden.tile([128, 1], F32, tag="den")
            nc.tensor.matmul(
                dp[:, :], et[:, :, sl], ones_col[:, :, :],
                start=True, stop=True, perf_mode=DR,
            )
            rr = work.tile([128, 1], F32, tag="rr")
            nc.vector.reciprocal(rr[:, :], dp[:, :])
            op = psum.tile([128, 512], F32, tag="mm")
            nc.tensor.matmul(
                op[:, :], et[:, :, sl], vw[:, :, :],
                start=True, stop=True, perf_mode=DR,
            )
            fin = finpool.tile([128, 512], F32, tag="fin")
            nc.vector.scalar_tensor_tensor(
                fin[:, :], op[:, :], rr[:, :], vstore[:, mi * 4 + t, :],
                op0=ALU.mult, op1=ALU.add,
            )
            nc.sync.dma_start(out=out_d[r0 : r0 + 128, :], in_=fin[:, :])


def build_nc(repeat: int = 1):
    nc = bacc.Bacc("TRN2", target_bir_lowering=False, debug=False, num_devices=8)
    xt_d = nc.declare_dram_parameter("xt", [CT, 128, N], BF16, isOutput=False)
    wqkv_d = nc.declare_dram_parameter("wqkv", [CT, 128, 3 * DIM], BF16, isOutput=False)
    wproj_d = nc.declare_dram_parameter("wproj", [CT, 128, DIM], BF16, isOutput=False)
    bproj_d = nc.declare_dram_parameter("bproj", [1, DIM], BF16, isOutput=False)
    out_d = nc.declare_dram_parameter("out", [N, DIM], F32, isOutput=True)
    aps = (out_d.ap(), xt_d.ap(), wqkv_d.ap(), wproj_d.ap(), bproj_d.ap())
    with tile.TileContext(nc) as tc, ExitStack() as ctx:
        if repeat == 1:
            build_kernel(ctx, tc, *aps)
        else:
            with tc.For_i(0, repeat, 1):
                build_kernel(ctx, tc, *aps)
    nc.compile()
    return nc


def prep_in_maps(x, W_qkv, W_proj, b_proj):
    bf = ml_dtypes.bfloat16
    wq = np.ascontiguousarray(np.asarray(W_qkv, np.float32).astype(bf).reshape(CT, 128, 3 * DIM))
    wp = np.ascontiguousarray(np.asarray(W_proj, np.float32).astype(bf).reshape(CT, 128, DIM))
    bp = np.asarray(b_proj, np.float32).astype(bf).reshape(1, DIM)
    in_maps = []
    for i in range(B):
        xti = np.ascontiguousarray(np.asarray(x[i], np.float32).T.astype(bf)).reshape(CT, 128, N)
        in_maps.append({"xt": xti, "wqkv": wq, "wproj": wp, "bproj": bp})
    return in_maps


_NC_CACHE = None


def kernel(x, W_qkv, W_proj, b_proj):
    global _NC_CACHE
    if _NC_CACHE is None:
        _NC_CACHE = build_nc()
    nc = _NC_CACHE
    in_maps = prep_in_maps(x, W_qkv, W_proj, b_proj)
    res = run_bass_kernel_spmd(nc, in_maps, core_ids=list(range(B)))
    out = np.stack([res.results[i]["out"] for i in range(B)], axis=0)
    return out.astype(np.float32)
